# revision 15
# baseline (speedup 1.0000x reference)
"""Trainium2 Bass kernel for nn_BlockFourierCirculant.

Math: y = irfft( einsum('oik,bik->bok', Wf, rfft(x.reshape(b,16,256))) )
with 4096 features = 16 blocks x 256 and a 129-bin half-spectrum.

The op is linear per batch row and factorizes into three matmul stages
(data-parallel over batch across 8 cores):

  stage 1: per input block i, forward real-DFT (256x256, matrix T1)
  stage 2: per frequency bin, a 32x32 real "complex multiply + block mix",
           bundled 4 bins at a time into 128x128 matrices (W2)
  stage 3: per output block o, inverse real-DFT (256x256, matrix B3)

Between stages the spectrum moves between batch-major and spectrum-major
partition layouts (the FFT butterfly).  Partition-crossing movement uses
the DMA xbar transpose, which requires 2-byte dtypes -- so x and the
spectrum intermediates travel as fp16 (5e-4 rounding; final accumulation
is always fp32 in PSUM, output y is fp32).

Per core / per batch-chunk of 256:
  s1  (x stationary):  out[b, slot]  = x[s, b].T @ T1[s, slot]
  xbar transpose:      (b, (g,i,u)) -> ((i,u), b)  per 4-bin group g
  s2  (X stationary):  out[b, (o,u')] = Xg[(i,u), b].T @ W2[(i,u), (o,u')]
  xbar transpose:      (b, (o,kch,slot)) -> (slot, b)
  s3  (B3 stationary): out[t, b]     = B3[slot, t].T @ Y[slot, b]
"""

import numpy as np

import concourse.bacc as bacc
import concourse.bass as bass
import concourse.mybir as mybir
import concourse.tile as tile
from concourse.bass_utils import run_bass_kernel_spmd

# ---------------------------------------------------------------- constants
BATCH = 8192
FEAT = 4096
BLOCK = 256
NBLK = 16  # blocks per row (in and out)
NSLOT = 256  # real spectrum slots per block
NGRP = 32  # stage-2 groups (8 slots each)
N_CORES = 8
BC = BATCH // N_CORES  # batch rows per core (1024)
NB = 256  # batch chunk
NCHUNK = BC // NB  # 4

F32 = mybir.dt.float32
F16 = mybir.dt.float16


# ------------------------------------------------------------- host matrices
def _slot_map():
    """slot s* in 0..255 per block -> (comp, bin); comp 0 = cos, 1 = sin."""
    m = [(0, 0), (0, 128)]
    for k in range(1, 128):
        m.append((0, k))
        m.append((1, k))
    return m


_SLOTS = _slot_map()


def build_t1():
    """(256 s, 256 slots) forward real-DFT, matching np.fft.rfft."""
    s = np.arange(BLOCK)
    T1 = np.zeros((BLOCK, NSLOT), dtype=np.float64)
    for j, (comp, k) in enumerate(_SLOTS):
        ang = 2.0 * np.pi * k * s / BLOCK
        T1[:, j] = np.cos(ang) if comp == 0 else -np.sin(ang)
    return T1


def build_b3():
    """(256 slots, 256 t) inverse real-DFT, matching np.fft.irfft."""
    t = np.arange(BLOCK)
    B3 = np.zeros((NSLOT, BLOCK), dtype=np.float64)
    for j, (comp, k) in enumerate(_SLOTS):
        w = 1.0 if k in (0, 128) else 2.0
        ang = 2.0 * np.pi * k * t / BLOCK
        B3[j] = (w * np.cos(ang) if comp == 0 else -w * np.sin(ang)) / BLOCK
    return B3


def build_w2(W_real, W_imag):
    """(32, 128, 128) per-group mixing matrices.

    Group g covers slots 8g..8g+7.  Row 8i+u reads Xhat[i, slot 8g+u];
    col 8o+u' writes Yhat[o, slot 8g+u'].
    """
    Wr = np.asarray(W_real, dtype=np.float64)
    Wi = np.asarray(W_imag, dtype=np.float64)
    W2 = np.zeros((NGRP, 128, 128), dtype=np.float64)
    for g in range(NGRP):
        for u in range(8):
            comp_u, k_u = _SLOTS[8 * g + u]
            for up in range(8):
                comp_up, k_up = _SLOTS[8 * g + up]
                if k_u != k_up:
                    continue
                k = k_u
                if comp_u == 0 and comp_up == 0:
                    coef = Wr[:, :, k]  # (o, i)
                elif comp_u == 1 and comp_up == 0:
                    coef = -Wi[:, :, k]
                elif comp_u == 0 and comp_up == 1:
                    coef = Wi[:, :, k]
                else:
                    coef = Wr[:, :, k]
                W2[g, u::8, up::8] = coef.T  # [i, o]
    return W2


def pack_t1(T1):
    """(128, 2, 256) fp16: [p, kc, slot] = T1[kc*128+p, slot]."""
    return np.ascontiguousarray(T1.reshape(2, 128, NSLOT).transpose(1, 0, 2)).astype(
        np.float16
    )


def pack_b3(B3):
    """(128, 2, 256) fp16: [p, kch, t] = B3[kch*128+p, t]."""
    return np.ascontiguousarray(B3.reshape(2, 128, BLOCK).transpose(1, 0, 2)).astype(
        np.float16
    )


def pack_w2(W2):
    """(128, 32, 128) fp16: [r, g, c] = W2[g, r, c]."""
    return np.ascontiguousarray(W2.transpose(1, 0, 2)).astype(np.float16)


# ------------------------------------------------------------- device kernel
def emit_kernel(tc, outs, ins, n_chunks=NCHUNK):
    """ins: xT (4096, n*NB) f16, t1 (128,2,256) f16, w2 (128,32,128) f16,
    b3 (128,2,256) f16;  outs: yT (4096, n*NB) f16."""
    nc = tc.nc
    bc = n_chunks * NB
    xT_r = ins["xT"].rearrange("(r p) b -> p r b", p=128)  # (128, 32, bc)
    yT_r = outs["yT"].rearrange("(r p) b -> p r b", p=128)

    with (
        tc.tile_pool(name="wpool", bufs=1) as wpool,
        tc.tile_pool(name="xpool", bufs=2) as xpool,
        tc.tile_pool(name="spec", bufs=3) as spec,
        tc.tile_pool(name="yst", bufs=2) as ystp,
        tc.tile_pool(name="psum", bufs=8, space="PSUM") as pspool,
    ):
        t1_sb = wpool.tile([128, 2, NSLOT], F16)
        w2_sb = wpool.tile([128, NGRP, 128], F16)
        b3_sb = wpool.tile([128, 2, BLOCK], F16)
        nc.sync.dma_start(out=t1_sb, in_=ins["t1"])
        nc.sync.dma_start(out=w2_sb, in_=ins["w2"])
        nc.sync.dma_start(out=b3_sb, in_=ins["b3"])

        ncopy = 0  # alternate evacuation copies between ACT and DVE

        def evac(dst, src):
            nonlocal ncopy
            if ncopy % 2 == 0:
                nc.scalar.copy(out=dst, in_=src)
            else:
                nc.vector.tensor_copy(out=dst, in_=src)
            ncopy += 1

        for n in range(n_chunks):
            bs = bass.ds(n * NB, NB)

            # ---- load x chunk [p, r, b] (fp16)
            x_sb = xpool.tile([128, 32, NB], F16, tag="x")
            nc.gpsimd.dma_start(out=x_sb, in_=xT_r[:, :, bs])

            # ---- stage 1 (x stationary): out (b x slot), batch-major
            # xbt[b, bsub, g, i, u] = Xhat[b', i, slot 8g+u]
            xbt = spec.tile([128, 2, NGRP, NBLK, 8], F16, tag="xbt")
            for i in range(0, NBLK, 2):
                for bsub in range(2):
                    ps = pspool.tile([128, 2, NSLOT], F32, tag="ps")
                    for ip in range(2):
                        for kc in range(2):
                            nc.tensor.matmul(
                                ps[:, ip, :],
                                x_sb[:, 2 * (i + ip) + kc, bsub * 128 : bsub * 128 + 128],
                                t1_sb[:, kc, :],
                                start=(kc == 0),
                                stop=(kc == 1),
                            )
                    evac(
                        xbt[:, bsub, :, i : i + 2, :],
                        ps.rearrange("p i (g u) -> p g i u", u=8),
                    )

            # ---- shuffle 1: batched xbar transpose to spectrum-major
            # xg[(i,u), bsub, g, b] = Xhat[b, i, slot 8g+u]
            xg = spec.tile([128, 2, NGRP, 128], F16, tag="xg")
            for bsub in range(2):
                nc.sync.dma_start_transpose(
                    out=xg[:, bsub, :, :], in_=xbt[:, bsub, :, :, :]
                )

            # ---- stage 2 (X stationary): out (b x (o,u')), batch-major
            # ybt[b, bsub, o, kch, g', u'] = Yhat[b', o, slot kch*128+8g'+u']
            ybt = spec.tile([128, 2, NBLK, 2, 16, 8], F16, tag="xbt")
            for g in range(0, NGRP, 4):
                kch, gp = divmod(g, 16)
                for bsub in range(2):
                    ps = pspool.tile([128, 4, 128], F32, tag="ps")
                    for q in range(4):
                        nc.tensor.matmul(
                            ps[:, q, :],
                            xg[:, bsub, g + q, :],
                            w2_sb[:, g + q, :],
                            start=True,
                            stop=True,
                        )
                    evac(
                        ybt[:, bsub, :, kch, gp : gp + 4, :],
                        ps.rearrange("p q (o u) -> p o q u", u=8),
                    )

            # ---- shuffle 2: batched xbar transpose to slot-major per block
            # yom[p4, bsub, o, kch, b] = Yhat[b, o, slot kch*128+p4]
            yom = spec.tile([128, 2, NBLK, 2, 128], F16, tag="xg")
            for bsub in range(2):
                nc.sync.dma_start_transpose(
                    out=yom[:, bsub, :, :, :], in_=ybt[:, bsub, :, :, :, :]
                )

            # ---- stage 3 (B3 stationary): out (t x b) + store (fp16)
            ybig = ystp.tile([128, 32, NB], F16, tag="ybig")
            for ob in range(0, NBLK, 4):
                pss = [
                    pspool.tile([128, 2, NB], F32, tag="ps", name=f"ps3_{ob}_{j}")
                    for j in range(4)
                ]
                for mch in range(2):
                    for kch in range(2):
                        for j in range(4):
                            nc.tensor.matmul(
                                pss[j][:, mch, :],
                                b3_sb[:, kch, mch * 128 : mch * 128 + 128],
                                yom[:, :, ob + j, kch, :],
                                start=(kch == 0),
                                stop=(kch == 1),
                            )
                for j in range(4):
                    evac(ybig[:, 2 * (ob + j) : 2 * (ob + j) + 2, :], pss[j])
            nc.gpsimd.dma_start(out=yT_r[:, :, bs], in_=ybig)


# ------------------------------------------------------------ host interface
_CACHED = {}


def make_inputs(W_real, W_imag):
    return {
        "t1": pack_t1(build_t1()),
        "w2": pack_w2(build_w2(W_real, W_imag)),
        "b3": pack_b3(build_b3()),
    }


def _build_bass():
    if "nc" in _CACHED:
        return _CACHED["nc"]
    nc = bacc.Bacc("TRN2", target_bir_lowering=False, debug=False)
    ins = {
        "xT": nc.dram_tensor("xT", [FEAT, BC], F16, kind="ExternalInput").ap(),
        "t1": nc.dram_tensor("t1", [128, 2, NSLOT], F16, kind="ExternalInput").ap(),
        "w2": nc.dram_tensor("w2", [128, NGRP, 128], F16, kind="ExternalInput").ap(),
        "b3": nc.dram_tensor("b3", [128, 2, BLOCK], F16, kind="ExternalInput").ap(),
    }
    outs = {"yT": nc.dram_tensor("yT", [FEAT, BC], F16, kind="ExternalOutput").ap()}
    with tile.TileContext(nc) as tc:
        emit_kernel(tc, outs, ins, NCHUNK)
    nc.compile()
    _CACHED["nc"] = nc
    return nc


def run_sharded(x, W_real, W_imag, trace=False):
    """Run on 8 NeuronCores; returns (y, BassKernelResults)."""
    x = np.asarray(x, dtype=np.float32)
    w = make_inputs(W_real, W_imag)

    in_maps = []
    for c in range(N_CORES):
        xT = np.ascontiguousarray(x[c * BC : (c + 1) * BC, :].T.astype(np.float16))
        in_maps.append({"xT": xT, **w})

    nc = _build_bass()
    res = run_bass_kernel_spmd(nc, in_maps, core_ids=list(range(N_CORES)), trace=trace)

    y = np.empty((BATCH, FEAT), dtype=np.float32)
    for c in range(N_CORES):
        y[c * BC : (c + 1) * BC, :] = res.results[c]["yT"].T.astype(np.float32)
    return y, res


def kernel(x, W_real, W_imag):
    y, _ = run_sharded(x, W_real, W_imag, trace=False)
    return y


# revision 16
# speedup vs baseline: 1.3908x; 1.3908x over previous
"""Trainium2 Bass kernel for nn_BlockFourierCirculant.

Math: y = irfft( einsum('oik,bik->bok', Wf, rfft(x.reshape(b,16,256))) )
with 4096 features = 16 blocks x 256 and a 129-bin half-spectrum.

The op is linear per batch row and factorizes into three matmul stages
(data-parallel over batch across 8 cores):

  stage 1: per input block i, forward real-DFT (256x256, matrix T1)
  stage 2: per frequency bin, a 32x32 real "complex multiply + block mix",
           bundled 4 bins at a time into 128x128 matrices (W2)
  stage 3: per output block o, inverse real-DFT (256x256, matrix B3)

Between stages the spectrum moves between batch-major and spectrum-major
partition layouts (the FFT butterfly).  Partition-crossing movement uses
the DMA xbar transpose, which requires 2-byte dtypes -- so x and the
spectrum intermediates travel as fp16 (5e-4 rounding; final accumulation
is always fp32 in PSUM, output y is fp32).

Per core / per batch-chunk of 256:
  s1  (x stationary):  out[b, slot]  = x[s, b].T @ T1[s, slot]
  xbar transpose:      (b, (g,i,u)) -> ((i,u), b)  per 4-bin group g
  s2  (X stationary):  out[b, (o,u')] = Xg[(i,u), b].T @ W2[(i,u), (o,u')]
  xbar transpose:      (b, (o,kch,slot)) -> (slot, b)
  s3  (B3 stationary): out[t, b]     = B3[slot, t].T @ Y[slot, b]
"""

import numpy as np

import concourse.bacc as bacc
import concourse.bass as bass
import concourse.mybir as mybir
import concourse.tile as tile
from concourse.bass_utils import run_bass_kernel_spmd

# ---------------------------------------------------------------- constants
BATCH = 8192
FEAT = 4096
BLOCK = 256
NBLK = 16  # blocks per row (in and out)
NSLOT = 256  # real spectrum slots per block
NGRP = 32  # stage-2 groups (8 slots each)
N_CORES = 8
BC = BATCH // N_CORES  # batch rows per core (1024)
NB = 256  # batch chunk
NCHUNK = BC // NB  # 4

F32 = mybir.dt.float32
F16 = mybir.dt.float16


# ------------------------------------------------------------- host matrices
def _slot_map():
    """slot s* in 0..255 per block -> (comp, bin); comp 0 = cos, 1 = sin."""
    m = [(0, 0), (0, 128)]
    for k in range(1, 128):
        m.append((0, k))
        m.append((1, k))
    return m


_SLOTS = _slot_map()


def build_t1():
    """(256 s, 256 slots) forward real-DFT, matching np.fft.rfft."""
    s = np.arange(BLOCK)
    T1 = np.zeros((BLOCK, NSLOT), dtype=np.float64)
    for j, (comp, k) in enumerate(_SLOTS):
        ang = 2.0 * np.pi * k * s / BLOCK
        T1[:, j] = np.cos(ang) if comp == 0 else -np.sin(ang)
    return T1


def build_b3():
    """(256 slots, 256 t) inverse real-DFT, matching np.fft.irfft."""
    t = np.arange(BLOCK)
    B3 = np.zeros((NSLOT, BLOCK), dtype=np.float64)
    for j, (comp, k) in enumerate(_SLOTS):
        w = 1.0 if k in (0, 128) else 2.0
        ang = 2.0 * np.pi * k * t / BLOCK
        B3[j] = (w * np.cos(ang) if comp == 0 else -w * np.sin(ang)) / BLOCK
    return B3


def build_w2(W_real, W_imag):
    """(32, 128, 128) per-group mixing matrices.

    Group g covers slots 8g..8g+7.  Row 8i+u reads Xhat[i, slot 8g+u];
    col 8o+u' writes Yhat[o, slot 8g+u'].
    """
    Wr = np.asarray(W_real, dtype=np.float64)
    Wi = np.asarray(W_imag, dtype=np.float64)
    W2 = np.zeros((NGRP, 128, 128), dtype=np.float64)
    for g in range(NGRP):
        for u in range(8):
            comp_u, k_u = _SLOTS[8 * g + u]
            for up in range(8):
                comp_up, k_up = _SLOTS[8 * g + up]
                if k_u != k_up:
                    continue
                k = k_u
                if comp_u == 0 and comp_up == 0:
                    coef = Wr[:, :, k]  # (o, i)
                elif comp_u == 1 and comp_up == 0:
                    coef = -Wi[:, :, k]
                elif comp_u == 0 and comp_up == 1:
                    coef = Wi[:, :, k]
                else:
                    coef = Wr[:, :, k]
                W2[g, u::8, up::8] = coef.T  # [i, o]
    return W2


def pack_t1(T1):
    """(128, 2, 256) fp16: [p, kc, slot] = T1[kc*128+p, slot]."""
    return np.ascontiguousarray(T1.reshape(2, 128, NSLOT).transpose(1, 0, 2)).astype(
        np.float16
    )


def pack_b3(B3):
    """(128, 2, 256) fp16: [p, kch, t] = B3[kch*128+p, t]."""
    return np.ascontiguousarray(B3.reshape(2, 128, BLOCK).transpose(1, 0, 2)).astype(
        np.float16
    )


def pack_w2(W2):
    """(128, 32, 128) fp16: [r, g, c] = W2[g, r, c]."""
    return np.ascontiguousarray(W2.transpose(1, 0, 2)).astype(np.float16)


# ------------------------------------------------------------- device kernel
def emit_kernel(tc, outs, ins, n_chunks=NCHUNK):
    """ins: xT (4096, n*NB) f16, t1 (128,2,256) f16, w2 (128,32,128) f16,
    b3 (128,2,256) f16;  outs: yT (4096, n*NB) f16."""
    nc = tc.nc
    bc = n_chunks * NB
    xT_r = ins["xT"].rearrange("(r p) b -> p r b", p=128)  # (128, 32, bc)
    yT_r = outs["yT"].rearrange("(r p) b -> p r b", p=128)

    with (
        tc.tile_pool(name="wpool", bufs=1) as wpool,
        tc.tile_pool(name="xpool", bufs=2) as xpool,
        tc.tile_pool(name="spec", bufs=3) as spec,
        tc.tile_pool(name="yst", bufs=2) as ystp,
        tc.tile_pool(name="psum", bufs=8, space="PSUM") as pspool,
    ):
        t1_sb = wpool.tile([128, 2, NSLOT], F16)
        w2_sb = wpool.tile([128, NGRP, 128], F16)
        b3_sb = wpool.tile([128, 2, BLOCK], F16)
        nc.sync.dma_start(out=t1_sb, in_=ins["t1"])
        nc.sync.dma_start(out=w2_sb, in_=ins["w2"])
        nc.sync.dma_start(out=b3_sb, in_=ins["b3"])

        ncopy = 0  # alternate evacuation copies between ACT and DVE

        def evac(dst, src):
            nonlocal ncopy
            if ncopy % 2 == 0:
                nc.scalar.copy(out=dst, in_=src)
            else:
                nc.vector.tensor_copy(out=dst, in_=src)
            ncopy += 1

        xg_t = {}
        yom_t = {}

        def stage1(n):
            """x load + per-block forward DFT + shuffle 1."""
            bs = bass.ds(n * NB, NB)
            x_sb = xpool.tile([128, 32, NB], F16, tag="x", name=f"x_{n}")
            nc.gpsimd.dma_start(out=x_sb, in_=xT_r[:, :, bs])

            # xbt[b, bsub, g, i, u] = Xhat[b', i, slot 8g+u]
            xbt = spec.tile([128, 2, NGRP, NBLK, 8], F16, tag="xbt", name=f"xbt_{n}")
            for i in range(0, NBLK, 2):
                for bsub in range(2):
                    ps = pspool.tile([128, 2, NSLOT], F32, tag="ps", name=f"ps1_{n}")
                    for ip in range(2):
                        for kc in range(2):
                            nc.tensor.matmul(
                                ps[:, ip, :],
                                x_sb[
                                    :, 2 * (i + ip) + kc, bsub * 128 : bsub * 128 + 128
                                ],
                                t1_sb[:, kc, :],
                                start=(kc == 0),
                                stop=(kc == 1),
                            )
                    evac(
                        xbt[:, bsub, :, i : i + 2, :],
                        ps.rearrange("p i (g u) -> p g i u", u=8),
                    )

            # shuffle 1: batched xbar transpose to spectrum-major
            # xg[(i,u), bsub, g, b] = Xhat[b, i, slot 8g+u]
            xg = spec.tile([128, 2, NGRP, 128], F16, tag="xg", name=f"xg_{n}")
            for bsub in range(2):
                nc.sync.dma_start_transpose(
                    out=xg[:, bsub, :, :], in_=xbt[:, bsub, :, :, :]
                )
            xg_t[n] = xg

        def stage2(n):
            """per-bin spectral mix + shuffle 2."""
            xg = xg_t.pop(n)
            # ybt[b, bsub, o, kch, g', u'] = Yhat[b', o, slot kch*128+8g'+u']
            ybt = spec.tile(
                [128, 2, NBLK, 2, 16, 8], F16, tag="xbt", name=f"ybt_{n}"
            )
            for g in range(0, NGRP, 4):
                kch, gp = divmod(g, 16)
                for bsub in range(2):
                    ps = pspool.tile([128, 4, 128], F32, tag="ps", name=f"ps2_{n}")
                    for q in range(4):
                        nc.tensor.matmul(
                            ps[:, q, :],
                            xg[:, bsub, g + q, :],
                            w2_sb[:, g + q, :],
                            start=True,
                            stop=True,
                        )
                    evac(
                        ybt[:, bsub, :, kch, gp : gp + 4, :],
                        ps.rearrange("p q (o u) -> p o q u", u=8),
                    )

            # shuffle 2: batched xbar transpose to slot-major per block
            # yom[p4, bsub, o, kch, b] = Yhat[b, o, slot kch*128+p4]
            yom = spec.tile([128, 2, NBLK, 2, 128], F16, tag="xg", name=f"yom_{n}")
            for bsub in range(2):
                nc.sync.dma_start_transpose(
                    out=yom[:, bsub, :, :, :], in_=ybt[:, bsub, :, :, :, :]
                )
            yom_t[n] = yom

        def stage3(n):
            """per-block inverse DFT + store (fp16)."""
            bs = bass.ds(n * NB, NB)
            yom = yom_t.pop(n)
            ybig = ystp.tile([128, 32, NB], F16, tag="ybig", name=f"ybig_{n}")
            for ob in range(0, NBLK, 4):
                pss = [
                    pspool.tile([128, 2, NB], F32, tag="ps", name=f"ps3_{n}_{ob}_{j}")
                    for j in range(4)
                ]
                for mch in range(2):
                    for kch in range(2):
                        for j in range(4):
                            nc.tensor.matmul(
                                pss[j][:, mch, :],
                                b3_sb[:, kch, mch * 128 : mch * 128 + 128],
                                yom[:, :, ob + j, kch, :],
                                start=(kch == 0),
                                stop=(kch == 1),
                            )
                for j in range(4):
                    evac(ybig[:, 2 * (ob + j) : 2 * (ob + j) + 2, :], pss[j])
            nc.gpsimd.dma_start(out=yT_r[:, :, bs], in_=ybig)

        # software-pipelined emission: PE keeps independent work in flight
        # while each chunk's shuffles and evacuations complete.
        for k in range(n_chunks + 2):
            if k < n_chunks:
                stage1(k)
            if 0 <= k - 1 < n_chunks:
                stage2(k - 1)
            if 0 <= k - 2 < n_chunks:
                stage3(k - 2)


# ------------------------------------------------------------ host interface
_CACHED = {}


def make_inputs(W_real, W_imag):
    return {
        "t1": pack_t1(build_t1()),
        "w2": pack_w2(build_w2(W_real, W_imag)),
        "b3": pack_b3(build_b3()),
    }


def _build_bass():
    if "nc" in _CACHED:
        return _CACHED["nc"]
    nc = bacc.Bacc("TRN2", target_bir_lowering=False, debug=False)
    ins = {
        "xT": nc.dram_tensor("xT", [FEAT, BC], F16, kind="ExternalInput").ap(),
        "t1": nc.dram_tensor("t1", [128, 2, NSLOT], F16, kind="ExternalInput").ap(),
        "w2": nc.dram_tensor("w2", [128, NGRP, 128], F16, kind="ExternalInput").ap(),
        "b3": nc.dram_tensor("b3", [128, 2, BLOCK], F16, kind="ExternalInput").ap(),
    }
    outs = {"yT": nc.dram_tensor("yT", [FEAT, BC], F16, kind="ExternalOutput").ap()}
    with tile.TileContext(nc) as tc:
        emit_kernel(tc, outs, ins, NCHUNK)
    nc.compile()
    _CACHED["nc"] = nc
    return nc


def run_sharded(x, W_real, W_imag, trace=False):
    """Run on 8 NeuronCores; returns (y, BassKernelResults)."""
    x = np.asarray(x, dtype=np.float32)
    w = make_inputs(W_real, W_imag)

    in_maps = []
    for c in range(N_CORES):
        xT = np.ascontiguousarray(x[c * BC : (c + 1) * BC, :].T.astype(np.float16))
        in_maps.append({"xT": xT, **w})

    nc = _build_bass()
    res = run_bass_kernel_spmd(nc, in_maps, core_ids=list(range(N_CORES)), trace=trace)

    y = np.empty((BATCH, FEAT), dtype=np.float32)
    for c in range(N_CORES):
        y[c * BC : (c + 1) * BC, :] = res.results[c]["yT"].T.astype(np.float32)
    return y, res


def kernel(x, W_real, W_imag):
    y, _ = run_sharded(x, W_real, W_imag, trace=False)
    return y
